# revision 1
# baseline (speedup 1.0000x reference)
"""Trainium2 Bass kernel for nn_Block_79680233275670 (dense transformer block).

Reference, for x [16, 1024, 384]:
  x = x + proj(attn(LN1(x)))                               (4 heads, head_dim 96)
  x = x + fc2(hswish(dw3x3(hswish(fc1(LN2(x))))))          (IRB, 32x32 spatial)

Sharding: pure data-parallel over batch B=16 -> 8 cores x 2 batch items.
No collectives. Weights replicated (pre-transposed / LN-folded / bf16 host-side).

Per-core dataflow (T = 2048 tokens = 2 batches x 1024):
  - x token-major [128, 16, 384] f32; residual stream stays f32
  - LN token-major (bn_stats) -> bf16, PE-transpose to channel-major;
    LN gamma/beta folded into downstream matmul weights host-side
  - q,k channel-major per head [96, T]; v token-major with an appended ones
    column (v_aug [.., 97]) so the PV matmul also emits softmax denominators
  - scores computed transposed St[m,n] = k^T q (so P needs no transpose);
    exp on ACT without max subtraction (|scores| < ~2); PV gives O_un [97, n];
    PE-transpose 128-token slices -> per-partition reciprocal + normalize
  - proj/fc2 token-major (activations as the stationary operand) so the f32
    residual adds are plain tensor ops; biases added via K=1 ones-row matmuls
  - IRB: fc1 channel-major; hardswish on DVE via min/max (the /6 folded into
    dw / fc2 weights); depthwise 3x3 as 9 shifted-AP multiply-accumulates
    split per channel-block across DVE / GPSIMD / PE(diagonal matmuls)
"""

import sys
import functools

for _p in ("/opt/trn_rl_repo",):
    if _p not in sys.path:
        sys.path.insert(0, _p)

import numpy as np
import ml_dtypes

import concourse.bass as bass
import concourse.mybir as mybir
import concourse.tile as tile
from concourse import bacc
from concourse.masks import make_identity

B, N, C = 16, 1024, 384
HEADS, HD = 4, 96
HID = 1536
NCORES = 8
BPC = B // NCORES          # batches per core
T = BPC * N                # tokens per core
NT = T // 128              # 16 token tiles per core
EPS = 1e-5

f32 = mybir.dt.float32
bf16 = mybir.dt.bfloat16
AF = mybir.ActivationFunctionType
OP = mybir.AluOpType
nbf = ml_dtypes.bfloat16

# engine per depthwise-conv chunk, indexed by hidden m-tile (12 per block):
# P = PE diag-matmul, D = DVE scalar_tensor_tensor, G = GPSIMD ts+tt pairs
DW_PATTERN = "PPDPPPDPPPDP"

WROWS = 17          # spatial rows per IRB window (16 out + 1 halo row)
WTOK = WROWS * 32   # 544
WP = 34             # padded row pitch (32 data + 2 zero pad cols = SAME x-padding)
HOFF = 2            # leading zero pad elems in h1 window tensors
HLEN = HOFF + WROWS * WP  # 580


def emit_kernel(nc, tc, d):
    from contextlib import ExitStack

    with ExitStack() as ctx:
        singles = ctx.enter_context(tc.tile_pool(name="singles", bufs=1))

        x_sb = singles.tile([128, NT, C], f32)   # token-major; becomes x2 in place
        ident = singles.tile([128, 128], bf16)
        make_identity(nc, ident)
        ones_row = singles.tile([1, 128], bf16)
        nc.vector.memset(ones_row, 1.0)
        eps_sb = singles.tile([128, 1], f32)
        nc.vector.memset(eps_sb, EPS)

        wqk_sb = singles.tile([128, 3, 2 * C], bf16)
        bqk_sb = singles.tile([96, 8], f32)
        wv_sb = singles.tile([128, 3, C], bf16)
        bv_sb = singles.tile([1, C], bf16)
        wp_sb = singles.tile([128, 3, C], bf16)
        bp_sb = singles.tile([1, C], bf16)
        wf1_sb = singles.tile([128, 3, HID], bf16)
        bf1_sb = singles.tile([128, 12], f32)
        wdw_sb = singles.tile([128, 12, 9], f32)
        bdw_sb = singles.tile([128, 12], f32)
        wf2_sb = singles.tile([128, 12, C], bf16)
        bf2_sb = singles.tile([1, C], bf16)

        for name, dst in (("wqk", wqk_sb), ("wv", wv_sb), ("wp", wp_sb),
                          ("wf1", wf1_sb)):
            nc.sync.dma_start(out=dst, in_=d[name].rearrange("k p m -> p k m"))
        nc.sync.dma_start(out=wf2_sb, in_=d["wf2"].rearrange("k p m -> p k m"))
        for name, dst in (("bqk", bqk_sb), ("bv", bv_sb), ("bp", bp_sb),
                          ("bf1", bf1_sb), ("wdw", wdw_sb), ("bdw", bdw_sb),
                          ("bf2", bf2_sb)):
            nc.sync.dma_start(out=dst, in_=d[name])

        # x[b, i*128+p, c] -> x_sb[p, b*8+i, c]
        nc.sync.dma_start(
            out=x_sb, in_=d["x"].rearrange("b (i p) c -> p (b i) c", p=128)
        )

        def layernorm_to_ch(xn_ch, ln_pool, ps_pool):
            for tt in range(NT):
                stats = ln_pool.tile([128, 6], f32, tag="ln_stats")
                nc.vector.bn_stats(stats, x_sb[:, tt, :])
                mv = ln_pool.tile([128, 2], f32, tag="ln_mv")
                nc.vector.bn_aggr(mv, stats)
                std = ln_pool.tile([128, 1], f32, tag="ln_std")
                nc.scalar.activation(std, mv[:, 1:2], AF.Sqrt, bias=eps_sb)
                rstd = ln_pool.tile([128, 1], f32, tag="ln_rstd")
                nc.vector.reciprocal(rstd, std)
                xn = ln_pool.tile([128, C], bf16, tag="ln_xn")
                nc.vector.tensor_scalar(
                    xn, x_sb[:, tt, :], mv[:, 0:1], rstd, OP.subtract, OP.mult
                )
                tp = ps_pool.tile([128, C], bf16, tag="ln_tp")
                for j in range(3):
                    nc.tensor.transpose(
                        tp[:, j * 128:(j + 1) * 128],
                        xn[:, j * 128:(j + 1) * 128], ident,
                    )
                nc.scalar.activation(
                    xn_ch[:, :, tt * 128:(tt + 1) * 128],
                    tp.rearrange("p (j t) -> p j t", j=3),
                    AF.Copy,
                )

        # ======================= attention =======================
        with tc.tile_pool(name="attn_acts", bufs=1) as apool:
            xn1_ch = apool.tile([128, 3, T], bf16)
            q_sb = apool.tile([96, HEADS, T], bf16)
            k_sb = apool.tile([96, HEADS, T], bf16)
            v_sb = apool.tile([128, NT, HEADS, HD + 1], bf16)
            o_norm = apool.tile([128, NT, HEADS, HD], bf16)
            o_ch = apool.tile([128, 3, T], bf16)
            nc.vector.memset(v_sb[:, :, :, HD:HD + 1], 1.0)

            with tc.tile_pool(name="ln1", bufs=3) as ln_pool, \
                 tc.tile_pool(name="ln1_ps", bufs=3, space="PSUM") as lnps_pool:
                layernorm_to_ch(xn1_ch, ln_pool, lnps_pool)

            with tc.tile_pool(name="qkv_ps", bufs=4, space="PSUM") as qkv_ps:
                for io in range(2):  # 0=q, 1=k
                    dst = q_sb if io == 0 else k_sb
                    for h in range(HEADS):
                        for cn in range(T // 512):
                            ps = qkv_ps.tile([96, 512], f32, tag="qk")
                            for kt in range(3):
                                nc.tensor.matmul(
                                    ps,
                                    wqk_sb[:, kt,
                                           io * C + h * HD: io * C + (h + 1) * HD],
                                    xn1_ch[:, kt, cn * 512:(cn + 1) * 512],
                                    start=(kt == 0), stop=(kt == 2),
                                )
                            nc.scalar.activation(
                                dst[:, h, cn * 512:(cn + 1) * 512], ps,
                                AF.Identity,
                                bias=bqk_sb[:, io * 4 + h: io * 4 + h + 1],
                            )
                for tt in range(NT):
                    ps = qkv_ps.tile([128, C], f32, tag="v")
                    for kt in range(3):
                        nc.tensor.matmul(
                            ps, xn1_ch[:, kt, tt * 128:(tt + 1) * 128],
                            wv_sb[:, kt, :], start=(kt == 0), stop=False,
                        )
                    nc.tensor.matmul(ps, ones_row, bv_sb, start=False, stop=True)
                    nc.scalar.activation(
                        v_sb[:, tt, :, 0:HD],
                        ps.rearrange("p (h e) -> p h e", h=HEADS),
                        AF.Copy,
                    )

            ou_tiles = {}
            with tc.tile_pool(name="st_ps", bufs=2, space="PSUM") as st_ps, \
                 tc.tile_pool(name="o_ps", bufs=2, space="PSUM") as o_ps, \
                 tc.tile_pool(name="pt_pool", bufs=4) as pt_pool, \
                 tc.tile_pool(name="ou_pool", bufs=1) as ou_pool:
                for b in range(BPC):
                    for h in range(HEADS):
                        o_psum = o_ps.tile([HD + 1, N], f32, tag="o")
                        for mt in range(8):
                            st = st_ps.tile([128, N], f32, tag="st")
                            for cn in range(2):
                                nc.tensor.matmul(
                                    st[:, cn * 512:(cn + 1) * 512],
                                    k_sb[:, h, b * N + mt * 128: b * N + (mt + 1) * 128],
                                    q_sb[:, h, b * N + cn * 512: b * N + (cn + 1) * 512],
                                    start=True, stop=True,
                                )
                            pt = pt_pool.tile([128, N], bf16, tag="pt")
                            nc.scalar.activation(pt, st, AF.Exp)
                            for cn in range(2):
                                nc.tensor.matmul(
                                    o_psum[:, cn * 512:(cn + 1) * 512],
                                    v_sb[:, b * 8 + mt, h, :],
                                    pt[:, cn * 512:(cn + 1) * 512],
                                    start=(mt == 0), stop=(mt == 7),
                                    skip_group_check=True,
                                )
                        # drain on DVE: ACT is the bottleneck in this phase
                        o_un = ou_pool.tile([HD + 1, N], bf16, tag=f"ou{b}{h}")
                        nc.vector.tensor_copy(o_un, o_psum)
                        ou_tiles[(b, h)] = o_un

            with tc.tile_pool(name="tp_ps", bufs=2, space="PSUM") as tp_ps, \
                 tc.tile_pool(name="r_pool", bufs=4) as r_pool, \
                 tc.tile_pool(name="ot_ps", bufs=3, space="PSUM") as ot_ps, \
                 tc.tile_pool(name="pj_ps", bufs=3, space="PSUM") as pj_ps:
                # deferred O normalization (transpose + per-token recip)
                for b in range(BPC):
                    for h in range(HEADS):
                        o_un = ou_tiles[(b, h)]
                        for ns in range(8):
                            tp = tp_ps.tile([128, HD + 1], bf16, tag="tp")
                            nc.tensor.transpose(
                                tp, o_un[:, ns * 128:(ns + 1) * 128],
                                ident[0:HD + 1, 0:HD + 1],
                            )
                            r = r_pool.tile([128, 1], f32, tag="r")
                            nc.vector.reciprocal(r, tp[:, HD:HD + 1])
                            nc.vector.tensor_scalar(
                                o_norm[:, b * 8 + ns, h, :], tp[:, 0:HD],
                                r, None, OP.mult,
                            )
                for tt in range(NT):
                    tp = ot_ps.tile([128, C], bf16, tag="ot")
                    ov = o_norm[:, tt, :, :].rearrange("p h e -> p (h e)")
                    for j in range(3):
                        nc.tensor.transpose(
                            tp[:, j * 128:(j + 1) * 128],
                            ov[:, j * 128:(j + 1) * 128], ident,
                        )
                    nc.scalar.activation(
                        o_ch[:, :, tt * 128:(tt + 1) * 128],
                        tp.rearrange("p (j t) -> p j t", j=3),
                        AF.Copy,
                    )
                for tt in range(NT):
                    ps = pj_ps.tile([128, C], f32, tag="pj")
                    for kt in range(3):
                        nc.tensor.matmul(
                            ps, o_ch[:, kt, tt * 128:(tt + 1) * 128],
                            wp_sb[:, kt, :], start=(kt == 0), stop=False,
                        )
                    nc.tensor.matmul(ps, ones_row, bp_sb, start=False, stop=True)
                    nc.vector.tensor_add(x_sb[:, tt, :], ps, x_sb[:, tt, :])

        # ======================= IRB branch =======================
        # depthwise layout: each window row padded to WP=34 cols; the 2 pad
        # cols are kept zero and provide the SAME zero-padding at x edges.
        ACCL = 16 * WP            # padded acc length (544)
        AUSE = ACCL - 2           # initialized acc prefix (542)
        with tc.tile_pool(name="irb_acts", bufs=1) as npool:
            xn2_ch = npool.tile([128, 3, T], bf16)
            h1w_a = npool.tile([128, 12, HLEN], bf16, tag="h1w_a")
            h1w_b = npool.tile([128, 12, HLEN], bf16, tag="h1w_b")
            h1w_bufs = [h1w_a, h1w_b]
            # zero once; interior writes never touch the pad columns again
            nc.gpsimd.memset(h1w_a, 0.0)
            nc.gpsimd.memset(h1w_b, 0.0)
            wdg_all = npool.tile([128, 12, 9, 128], bf16)
            nc.sync.dma_start(
                out=wdg_all, in_=d["wdiag"].rearrange("m t c j -> c m t j"))

            with tc.tile_pool(name="ln2", bufs=3) as ln_pool, \
                 tc.tile_pool(name="ln2_ps", bufs=3, space="PSUM") as lnps_pool:
                layernorm_to_ch(xn2_ch, ln_pool, lnps_pool)

            with tc.tile_pool(name="h2_pool", bufs=2) as h2_pool, \
                 tc.tile_pool(name="hs_pool", bufs=3) as hs_pool, \
                 tc.tile_pool(name="dwa_pool", bufs=3) as dwa_pool, \
                 tc.tile_pool(name="out_pool", bufs=4) as out_pool, \
                 tc.tile_pool(name="f1_ps", bufs=2, space="PSUM") as f1_ps, \
                 tc.tile_pool(name="dw_ps", bufs=1, space="PSUM") as dw_ps, \
                 tc.tile_pool(name="f2_ps", bufs=2, space="PSUM") as f2_ps:
                for b in range(BPC):
                    for yh in range(2):
                        r0 = yh * 16               # first output spatial row
                        wy0 = 0 if yh == 0 else 15  # first window row
                        tok0 = b * N + wy0 * 32
                        h1w = h1w_bufs[(b * 2 + yh) % 2]
                        for m in range(12):
                            ps = f1_ps.tile([128, WTOK], f32, tag="f1")
                            for c0, cw in ((0, 512), (512, WTOK - 512)):
                                for kt in range(3):
                                    nc.tensor.matmul(
                                        ps[:, c0:c0 + cw],
                                        wf1_sb[:, kt, m * 128:(m + 1) * 128],
                                        xn2_ch[:, kt, tok0 + c0: tok0 + c0 + cw],
                                        start=(kt == 0), stop=(kt == 2),
                                    )
                            # hardswish 1 (its /6 is folded into wdw)
                            v1 = hs_pool.tile([128, WTOK], bf16, tag="v1")
                            nc.scalar.activation(
                                v1, ps, AF.Identity, bias=bf1_sb[:, m:m + 1]
                            )
                            t1 = hs_pool.tile([128, WTOK], bf16, tag="t1")
                            nc.vector.tensor_scalar(t1, v1, 3.0, 0.0, OP.add, OP.max)
                            h1v = h1w[:, m, HOFF:HOFF + WROWS * WP].rearrange(
                                "p (y x) -> p y x", x=WP)[:, :, 0:32]
                            nc.vector.scalar_tensor_tensor(
                                h1v,
                                t1.rearrange("p (y x) -> p y x", x=32), 6.0,
                                v1.rearrange("p (y x) -> p y x", x=32),
                                OP.min, OP.mult,
                            )
                        for m in range(12):
                            eng = DW_PATTERN[m]
                            taps = []
                            for dy in (-1, 0, 1):
                                for dx in (-1, 0, 1):
                                    ti = (dy + 1) * 3 + (dx + 1)
                                    y0 = max(r0, -dy)           # first valid out row
                                    y1 = min(r0 + 16, 32 - dy)  # past-last out row
                                    ay = y0 - r0
                                    cy = y1 - y0
                                    sy = y0 + dy - wy0          # window-local src row
                                    taps.append((ti, dx, ay, cy, sy))
                            taps.sort(key=lambda t: (t[0] != 4, t[0]))
                            if eng == "P":
                                wdg = wdg_all[:, m]
                                dps = dw_ps.tile([128, ACCL], f32, tag="dwp")
                                BANK = 512  # f32 elems per PSUM bank
                                for i, (ti, dx, ay, cy, sy) in enumerate(taps):
                                    L = cy * WP - 2
                                    so = HOFF + sy * WP + dx
                                    a0 = ay * WP
                                    # a matmul may not cross a PSUM bank boundary
                                    cuts = [0]
                                    if a0 < BANK < a0 + L:
                                        cuts.append(BANK - a0)
                                    cuts.append(L)
                                    for ci in range(len(cuts) - 1):
                                        u0, u1 = cuts[ci], cuts[ci + 1]
                                        nc.tensor.matmul(
                                            dps[:, a0 + u0: a0 + u1],
                                            wdg[:, ti, :],
                                            h1w[:, m, so + u0: so + u1],
                                            start=(i == 0),
                                            stop=(i == len(taps) - 1 and ci == len(cuts) - 2),
                                            skip_group_check=True,
                                        )
                                v2 = hs_pool.tile([128, ACCL], bf16, tag="v2")
                                nc.scalar.activation(
                                    v2[:, 0:AUSE], dps[:, 0:AUSE], AF.Identity,
                                    bias=bdw_sb[:, m:m + 1],
                                )
                            else:
                                e = nc.vector if eng == "D" else nc.gpsimd
                                acc = dwa_pool.tile([128, ACCL], bf16, tag="dwa")
                                for i, (ti, dx, ay, cy, sy) in enumerate(taps):
                                    L = cy * WP - 2
                                    so = HOFF + sy * WP + dx
                                    src = h1w[:, m, so: so + L]
                                    av = acc[:, ay * WP: ay * WP + L]
                                    wsc = wdw_sb[:, m, ti:ti + 1]
                                    if i == 0:
                                        e.tensor_scalar(av, src, wsc, None, OP.mult)
                                    elif eng == "G":
                                        # Pool rejects scalar_tensor_tensor;
                                        # use a mult + add pair instead
                                        tmp = dwa_pool.tile([128, ACCL], bf16,
                                                            tag="dwt")
                                        tv = tmp[:, ay * WP: ay * WP + L]
                                        e.tensor_scalar(tv, src, wsc, None, OP.mult)
                                        e.tensor_tensor(out=av, in0=av, in1=tv,
                                                        op=OP.add)
                                    else:
                                        e.scalar_tensor_tensor(
                                            av, src, wsc, av, OP.mult, OP.add
                                        )
                                v2 = hs_pool.tile([128, ACCL], bf16, tag="v2")
                                nc.vector.tensor_scalar(
                                    v2[:, 0:AUSE], acc[:, 0:AUSE],
                                    bdw_sb[:, m:m + 1], None, OP.add,
                                )
                            t2 = hs_pool.tile([128, ACCL], bf16, tag="t2")
                            nc.vector.tensor_scalar(
                                t2[:, 0:AUSE], v2[:, 0:AUSE], 3.0, 0.0,
                                OP.add, OP.max,
                            )
                            if m == 0:
                                h2 = h2_pool.tile([128, 12, 512], bf16, tag="h2")
                            pv = lambda a: a.rearrange(
                                "p (y x) -> p y x", x=WP)[:, :, 0:32]
                            nc.vector.scalar_tensor_tensor(
                                h2[:, m, :].rearrange("p (y x) -> p y x", x=32),
                                pv(t2), 6.0, pv(v2), OP.min, OP.mult,
                            )
                        # fc2 + residual (hswish2's /6 folded into wf2)
                        for tl in range(4):
                            tg = b * 8 + yh * 4 + tl
                            ps = f2_ps.tile([128, C], f32, tag="f2")
                            for m in range(12):
                                nc.tensor.matmul(
                                    ps, h2[:, m, tl * 128:(tl + 1) * 128],
                                    wf2_sb[:, m, :], start=(m == 0), stop=False,
                                )
                            nc.tensor.matmul(ps, ones_row, bf2_sb,
                                             start=False, stop=True)
                            ot = out_pool.tile([128, C], f32, tag="out")
                            nc.vector.tensor_add(ot, ps, x_sb[:, tg, :])
                            nc.sync.dma_start(
                                out=d["out"][b,
                                             (yh * 4 + tl) * 128:(yh * 4 + tl + 1) * 128,
                                             :],
                                in_=ot,
                            )


def declare_tensors(nc):
    d = {}
    d["x"] = nc.dram_tensor("x", [BPC, N, C], f32, kind="ExternalInput").ap()
    d["wqk"] = nc.dram_tensor("wqk", [3, 128, 2 * C], bf16, kind="ExternalInput").ap()
    d["bqk"] = nc.dram_tensor("bqk", [96, 8], f32, kind="ExternalInput").ap()
    d["wv"] = nc.dram_tensor("wv", [3, 128, C], bf16, kind="ExternalInput").ap()
    d["bv"] = nc.dram_tensor("bv", [1, C], bf16, kind="ExternalInput").ap()
    d["wp"] = nc.dram_tensor("wp", [3, 128, C], bf16, kind="ExternalInput").ap()
    d["bp"] = nc.dram_tensor("bp", [1, C], bf16, kind="ExternalInput").ap()
    d["wf1"] = nc.dram_tensor("wf1", [3, 128, HID], bf16, kind="ExternalInput").ap()
    d["bf1"] = nc.dram_tensor("bf1", [128, 12], f32, kind="ExternalInput").ap()
    d["wdw"] = nc.dram_tensor("wdw", [128, 12, 9], f32, kind="ExternalInput").ap()
    d["wdiag"] = nc.dram_tensor("wdiag", [12, 9, 128, 128], bf16,
                                kind="ExternalInput").ap()
    d["bdw"] = nc.dram_tensor("bdw", [128, 12], f32, kind="ExternalInput").ap()
    d["wf2"] = nc.dram_tensor("wf2", [12, 128, C], bf16, kind="ExternalInput").ap()
    d["bf2"] = nc.dram_tensor("bf2", [1, C], bf16, kind="ExternalInput").ap()
    d["out"] = nc.dram_tensor("out", [BPC, N, C], f32, kind="ExternalOutput").ap()
    return d


@functools.lru_cache(maxsize=1)
def build_program(num_devices=NCORES):
    nc = bacc.Bacc("TRN2", target_bir_lowering=False, debug=False,
                   num_devices=num_devices)
    d = declare_tensors(nc)
    with tile.TileContext(nc) as tc:
        emit_kernel(nc, tc, d)
    nc.compile()
    return nc


def prep_weights(inputs):
    """Host-side packing: transposes, LN folds, bf16 casts."""
    g1 = np.asarray(inputs["ln1_g"], np.float32)
    b1 = np.asarray(inputs["ln1_b"], np.float32)
    g2 = np.asarray(inputs["ln2_g"], np.float32)
    b2 = np.asarray(inputs["ln2_b"], np.float32)
    Wqkv = np.asarray(inputs["Wqkv"], np.float32)
    Wproj = np.asarray(inputs["Wproj"], np.float32)
    bproj = np.asarray(inputs["bproj"], np.float32)
    Wfc1 = np.asarray(inputs["Wfc1"], np.float32)[:, :, 0, 0]
    bfc1 = np.asarray(inputs["bfc1"], np.float32)
    Wdw = np.asarray(inputs["Wdw"], np.float32)[:, 0].reshape(HID, 9)
    bdw = np.asarray(inputs["bdw"], np.float32)
    Wfc2 = np.asarray(inputs["Wfc2"], np.float32)[:, :, 0, 0]
    bfc2 = np.asarray(inputs["bfc2"], np.float32)

    W3 = Wqkv.reshape(HEADS, 3, HD, C)      # out channel o = h*288 + s*96 + d
    scale = float(HD) ** -0.5
    Wq = W3[:, 0].reshape(HEADS * HD, C)
    Wk = W3[:, 1].reshape(HEADS * HD, C)
    Wv = W3[:, 2].reshape(HEADS * HD, C)

    d = {}
    d["wqk"] = np.ascontiguousarray(
        np.concatenate([Wq * g1[None, :] * scale, Wk * g1[None, :]], 0).T
        .reshape(3, 128, 2 * C)).astype(nbf)
    d["bqk"] = np.ascontiguousarray(np.concatenate(
        [((Wq @ b1) * scale).reshape(HEADS, HD).T,
         (Wk @ b1).reshape(HEADS, HD).T], 1)).astype(np.float32)
    d["wv"] = np.ascontiguousarray(
        (Wv * g1[None, :]).T.reshape(3, 128, C)).astype(nbf)
    d["bv"] = (Wv @ b1)[None, :].astype(nbf)
    d["wp"] = np.ascontiguousarray(Wproj.T.reshape(3, 128, C)).astype(nbf)
    d["bp"] = bproj[None, :].astype(nbf)
    d["wf1"] = np.ascontiguousarray(
        (Wfc1 * g2[None, :]).T.reshape(3, 128, HID)).astype(nbf)
    d["bf1"] = np.ascontiguousarray(
        (bfc1 + Wfc1 @ b2).reshape(12, 128).T).astype(np.float32)
    wdw_full = Wdw / 6.0
    d["wdw"] = np.ascontiguousarray(
        wdw_full.reshape(12, 128, 9).transpose(1, 0, 2)).astype(np.float32)
    wdiag = np.zeros((12, 9, 128, 128), np.float32)
    ii = np.arange(128)
    for m in range(12):
        for t in range(9):
            wdiag[m, t, ii, ii] = wdw_full[m * 128 + ii, t]
    d["wdiag"] = wdiag.astype(nbf)
    d["bdw"] = np.ascontiguousarray(bdw.reshape(12, 128).T).astype(np.float32)
    d["wf2"] = np.ascontiguousarray((Wfc2 / 6.0).T.reshape(12, 128, C)).astype(nbf)
    d["bf2"] = bfc2[None, :].astype(nbf)
    return d


def kernel(**inputs):
    from concourse.bass_utils import run_bass_kernel_spmd

    x = np.asarray(inputs["x"], np.float32)
    wd = prep_weights(inputs)
    nc = build_program()
    in_maps = []
    for c in range(NCORES):
        m = dict(wd)
        m["x"] = np.ascontiguousarray(x[c * BPC:(c + 1) * BPC])
        in_maps.append(m)
    res = run_bass_kernel_spmd(nc, in_maps, list(range(NCORES)))
    out = np.concatenate([res.results[c]["out"] for c in range(NCORES)], axis=0)
    return out.astype(np.float32)

